# revision 1
# baseline (speedup 1.0000x reference)
"""Multi-head causal self-attention (B=2, L=2048, D=1024, H=16) on 8 TRN2
NeuronCores.

Sharding: core c handles batch b = c // 4 and head group g = c % 4 (4 heads,
i.e. a 256-wide slice of the QKV output dim and the matching 256 rows of
Wo^T).  Each core computes a full (L, D) partial of the output projection;
the host sums the 4 partials per batch and adds bo.

On-core layout (f32 everywhere, f32r = tf32 for matmul inputs):
  XT  [128, 8, 2048]   x^T  (d-chunk on partitions), scoped pool, via PE
  WqT/WkT/WvT [128, 8, 256]  W^T, scoped pool, via PE transpose
  QT/KT [128, 2, 2048] q^T / k^T (dq on partitions, chunk = head pair)
  KTz [128, 4, 2048]   k^T zero-padded per head to K=128 rows: the PE HAM
                       clock gate only un-throttles (1.2 -> 2.4 GHz) when
                       matmuls stream all 128 partitions, and K=64 f32r
                       matmuls are also ~1.6x slower per column (measured).
  Vp  [128, 16, 4, 65] v natural + ones column (softmax denominator trick)
  OT  [128, 2, 2048]   attention out^T, normalized in place
  WoT [128, 2, 1024]   Wo^T slice (built after the projection pool closes)

Attention per (qt, head): s^T[k, q] = KTz_h . QT_pair with zero rows killing
the other head; exp on ACT from a 2-bank PSUM pair; causal mask via gpsimd
affine_select on diagonal tiles; o^T + denominator accumulated in PSUM with
V'; normalize = PE ones-broadcast of the denominator + DVE reciprocal of the
broadcast (64 parallel lanes) + one multiply, emitted one tile late so the
PE stream never waits. Output projection is woven in per 512-row q block.
"""

import sys

for _p in ("/opt/trn_rl_repo", "/root/.axon_site/_ro/trn_rl_repo"):
    if _p not in sys.path:
        sys.path.append(_p)

from contextlib import ExitStack

import numpy as np

import concourse.bass as bass
import concourse.tile as tile
from concourse import bacc, mybir
from concourse.bass_utils import run_bass_kernel_spmd
from concourse.masks import make_identity

F32 = mybir.dt.float32
F32R = mybir.dt.float32r
F16 = mybir.dt.float16

B, L, D, H = 2, 2048, 1024, 16
DK = D // H  # 64
NCORES = 8
GH = 4  # heads per core
C = GH * DK  # 256: per-core slice of the qkv/head dim
QT_TILES = L // 512  # 4
KT_TILES = L // 128  # 16
DCH = D // 128  # 8


def _transpose_block(nc, psum, dst, w_sb, ident, n_chunks, col0):
    """PE-transpose `n_chunks` 128x128 blocks of w_sb into dst[:, chunk, col0:]."""
    for half in range(0, n_chunks, 4):
        n = min(4, n_chunks - half)
        pt = psum.tile([128, 512], F32, tag="ps", name="pt")
        for i in range(n):
            dci = half + i
            nc.tensor.matmul(
                pt[:, i * 128 : (i + 1) * 128],
                lhsT=w_sb[:, dci * 128 : (dci + 1) * 128],
                rhs=ident[:],
                is_transpose=True,
                start=(i == 0),
                stop=(i == n - 1),
            )
        nc.vector.tensor_copy(
            dst[:, half : half + n, col0 : col0 + 128],
            pt[:, : n * 128].rearrange("p (c q) -> p c q", c=n),
        )


def _build_program():
    nc = bacc.Bacc("TRN2", target_bir_lowering=False, debug=False, num_devices=NCORES)

    x_d = nc.dram_tensor("x", [L, D], F32, kind="ExternalInput").ap()
    wq_d = nc.dram_tensor("wq", [C, D], F32, kind="ExternalInput").ap()
    wk_d = nc.dram_tensor("wk", [C, D], F32, kind="ExternalInput").ap()
    wv_d = nc.dram_tensor("wv", [C, D], F32, kind="ExternalInput").ap()
    wo_d = nc.dram_tensor("wo", [D, C], F32, kind="ExternalInput").ap()
    bq_d = nc.dram_tensor("bq", [C], F32, kind="ExternalInput").ap()
    bk_d = nc.dram_tensor("bk", [C], F32, kind="ExternalInput").ap()
    bv_d = nc.dram_tensor("bv", [C], F32, kind="ExternalInput").ap()
    out_d = nc.dram_tensor("out", [L, D], F32, kind="ExternalOutput").ap()

    with tile.TileContext(nc) as tc, ExitStack() as ctx:
        pool = ctx.enter_context(tc.tile_pool(name="persist", bufs=1))
        psum = ctx.enter_context(tc.tile_pool(name="psum", bufs=4, space="PSUM"))
        psum2 = ctx.enter_context(tc.tile_pool(name="psum2", bufs=2, space="PSUM"))
        cp = ctx.enter_context(tc.tile_pool(name="copies", bufs=3))

        ident = pool.tile([128, 128], F32)
        make_identity(nc, ident)

        ones_f32 = pool.tile([128, 128], F32)
        nc.gpsimd.memset(ones_f32[:], 1.0)
        ones_r = pool.tile([1, 128], F32R)
        nc.vector.tensor_copy(ones_r[:], ones_f32[0:1, :])

        bq_sb = pool.tile([128, 2], F32)
        nc.sync.dma_start(bq_sb[:], bq_d.rearrange("(c p) -> p c", p=128))
        bk_sb = pool.tile([128, 2], F32)
        nc.sync.dma_start(bk_sb[:], bk_d.rearrange("(c p) -> p c", p=128))
        bv_sb = pool.tile([1, 256], F32)
        nc.sync.dma_start(bv_sb[:], bv_d[None, :])
        bv_r = pool.tile([1, 256], F32R)
        nc.vector.tensor_copy(bv_r[:], bv_sb[:])

        QTs = [pool.tile([128, 2, 512], F16, name=f"QT{g}") for g in range(4)]
        KTzs = [pool.tile([128, GH, 512], F16, name=f"KTz{g}") for g in range(4)]
        zeros_f32 = pool.tile([64, 512], F32)
        nc.gpsimd.memset(zeros_f32[:], 0.0)
        for g in range(4):
            for h in range(GH):
                zp = 64 - 64 * (h % 2)
                nc.vector.tensor_copy(KTzs[g][zp : zp + 64, h, :], zeros_f32[:])
        Vps = [pool.tile([128, 4, GH, DK + 1], F16, name=f"Vp{g}") for g in range(4)]
        OTs = [pool.tile([128, 2, 512], F32R, name=f"OT{g}") for g in range(4)]

        with nc.allow_low_precision(reason="tf32 rounding for f32r matmul inputs"):
            # ======== phase A (scoped pool): x^T, W^T, q/k/v projections ====
            with tc.tile_pool(name="projs", bufs=1) as pp, tc.tile_pool(
                name="loads", bufs=4
            ) as lp:
                XT = pp.tile([128, DCH, L], F32R)
                for qt in range(KT_TILES):  # 16 q-tiles of 128
                    x_sb = lp.tile([128, D], F32, tag="ld")
                    for hh in range(2):
                        nc.sync.dma_start(
                            x_sb[:, hh * 512 : (hh + 1) * 512],
                            x_d[qt * 128 : (qt + 1) * 128, hh * 512 : (hh + 1) * 512],
                        )
                    _transpose_block(nc, psum, XT, x_sb, ident, DCH, qt * 128)

                WT = {}
                for name, w_d in (("q", wq_d), ("k", wk_d), ("v", wv_d)):
                    wt = pp.tile([128, DCH, C], F32R, name=f"W{name}T")
                    WT[name] = wt
                    for j in range(2):  # dq chunks of 128
                        w_sb = lp.tile([128, D], F32, tag="ld")
                        nc.sync.dma_start(w_sb[:], w_d[j * 128 : (j + 1) * 128, :])
                        _transpose_block(nc, psum, wt, w_sb, ident, DCH, j * 128)

                WoT = pool.tile([128, 2, D], F32R)
                for ei in range(DCH):  # 8 chunks of e (output dim)
                    w_sb = lp.tile([128, C], F32, tag="ldo", bufs=2)
                    nc.sync.dma_start(w_sb[:], wo_d[ei * 128 : (ei + 1) * 128, :])
                    _transpose_block(nc, psum, WoT, w_sb, ident, 2, ei * 128)

                # ---- Q^T, K^T projections ----
                for name, wt, b_sb in (("q", WT["q"], bq_sb), ("k", WT["k"], bk_sb)):
                    for j in range(2):
                        for qt in range(QT_TILES):
                            ps = psum.tile([128, 512], F32, tag="ps")
                            for dci in range(DCH):
                                nc.tensor.matmul(
                                    ps[:],
                                    lhsT=wt[:, dci, j * 128 : (j + 1) * 128],
                                    rhs=XT[:, dci, qt * 512 : (qt + 1) * 512],
                                    start=(dci == 0),
                                    stop=(dci == DCH - 1),
                                )
                            if name == "q":
                                nc.vector.tensor_tensor(
                                    QTs[qt][:, j, :],
                                    ps[:],
                                    b_sb[:, j, None].to_broadcast((128, 512)),
                                    mybir.AluOpType.add,
                                )
                            else:
                                # write k^T split into the zero-padded KTz
                                for half in range(2):
                                    hp = 64 * half
                                    nc.vector.tensor_tensor(
                                        KTzs[qt][hp : hp + 64, 2 * j + half, :],
                                        ps[hp : hp + 64, :],
                                        b_sb[hp : hp + 64, j, None].to_broadcast(
                                            (64, 512)
                                        ),
                                        mybir.AluOpType.add,
                                    )

                # ---- V projection (natural layout + ones col) ----
                for kt in range(KT_TILES):
                    ps = psum.tile([128, 512], F32, tag="ps")
                    for dci in range(DCH):
                        nc.tensor.matmul(
                            ps[:, :256],
                            lhsT=XT[:, dci, kt * 128 : (kt + 1) * 128],
                            rhs=WT["v"][:, dci, :],
                            start=(dci == 0),
                            stop=False,
                        )
                    nc.tensor.matmul(
                        ps[:, :256], lhsT=ones_r[:], rhs=bv_r[:], start=False, stop=True
                    )
                    nc.vector.tensor_copy(
                        Vps[kt // 4][:, kt % 4, :, 0:DK],
                        ps[:, :256].rearrange("p (h d) -> p h d", h=GH),
                    )
                for g in range(4):
                    nc.vector.tensor_copy(
                        Vps[g][:, :, :, DK : DK + 1], ones_f32[:, 0:16]
                    )

            yp = ctx.enter_context(tc.tile_pool(name="youts", bufs=2))


            # ======== phase C: attention + woven output projection ========
            def normalize(h, qt, pso):
                hj, hp = h // 2, 64 * (h % 2)
                den_r = cp.tile([1, 512], F32R, tag="den", bufs=2)
                nc.vector.tensor_copy(den_r[:], pso[64:65, :])
                psb = psum.tile([128, 512], F32, tag="ps")
                nc.tensor.matmul(
                    psb[:64], lhsT=ones_r[:, 0:64], rhs=den_r[:], start=True, stop=True
                )
                rb = cp.tile([64, 512], F32, tag="rb", bufs=2)
                nc.vector.reciprocal(rb[:], psb[:64])
                nc.vector.tensor_tensor(
                    OTs[qt][hp : hp + 64, hj, :],
                    pso[:64],
                    rb[:],
                    mybir.AluOpType.mult,
                )

            def outproj(qt512):
                # project q rows [qt512*512, +512) and DMA them out; woven
                # into the next q-tile's attention so PE stays dense and the
                # output DMA is spread across the kernel.
                for sub in range(4):
                    q0 = qt512 * 512 + sub * 128
                    for e in range(2):
                        psy = psum.tile([128, 512], F32, tag="ps")
                        for cj in range(2):
                            nc.tensor.matmul(
                                psy[:],
                                lhsT=OTs[qt512][:, cj, sub * 128 : (sub + 1) * 128],
                                rhs=WoT[:, cj, e * 512 : (e + 1) * 512],
                                start=(cj == 0),
                                stop=(cj == 1),
                            )
                        y_sb = yp.tile([128, 512], F32, tag="y")
                        nc.vector.tensor_copy(y_sb[:], psy[:])
                        nc.sync.dma_start(
                            out_d[q0 : q0 + 128, e * 512 : (e + 1) * 512], y_sb[:]
                        )

            pending = None
            for qt in range(QT_TILES):
                n_kt = 4 * qt + 4
                for h in range(GH):
                    hj, hp = h // 2, 64 * (h % 2)
                    pso = psum.tile([128, 512], F32, tag="ps")
                    for kt2 in range(n_kt // 2):  # k-tile pairs share a
                        pss = psum2.tile([128, 1024], F32, tag="ps2")  # 2-bank psum
                        for i in range(2):
                            kt = 2 * kt2 + i
                            nc.tensor.matmul(
                                pss[:, i * 512 : (i + 1) * 512],
                                lhsT=KTzs[kt // 4][
                                    :, h, (kt % 4) * 128 : (kt % 4 + 1) * 128
                                ],
                                rhs=QTs[qt][:, hj, :],
                                start=True,
                                stop=True,
                            )
                        p_sb = cp.tile([128, 1024], F16, tag="p", bufs=5)
                        nc.scalar.activation(
                            p_sb[:],
                            pss[:],
                            mybir.ActivationFunctionType.Exp,
                            scale=0.125,
                        )
                        for i in range(2):
                            kt = 2 * kt2 + i
                            if kt >= 4 * qt:  # diagonal overlap: causal mask
                                nc.gpsimd.affine_select(
                                    out=p_sb[:, i * 512 : (i + 1) * 512],
                                    in_=p_sb[:, i * 512 : (i + 1) * 512],
                                    pattern=[[1, 512]],
                                    compare_op=mybir.AluOpType.is_ge,
                                    fill=0.0,
                                    base=qt * 512 - kt * 128,
                                    channel_multiplier=-1,
                                )
                            nc.tensor.matmul(
                                pso[:65],
                                lhsT=Vps[kt // 4][:, kt % 4, h, :],
                                rhs=p_sb[:, i * 512 : (i + 1) * 512],
                                start=(kt == 0),
                                stop=(kt == n_kt - 1),
                            )
                        if kt2 == 0 and pending is not None:
                            normalize(*pending)  # previous tile, PE has work
                            pending = None
                    pending = (h, qt, pso)
                    if h == 0 and qt > 0:
                        outproj(qt - 1)  # previous q block fully normalized
            normalize(*pending)
            outproj(QT_TILES - 1)

    nc.compile()
    return nc


_NC_CACHE = None


def _get_program():
    global _NC_CACHE
    if _NC_CACHE is None:
        _NC_CACHE = _build_program()
    return _NC_CACHE


def _run(in_maps, trace=False, **kw):
    nc = _get_program()
    return run_bass_kernel_spmd(nc, in_maps, list(range(NCORES)), trace=trace, **kw)


def _make_in_maps(x, Wq, bq, Wk, bk, Wv, bv, Wo, bo):
    a = lambda v: np.ascontiguousarray(np.asarray(v, dtype=np.float32))
    in_maps = []
    for core in range(NCORES):
        b, g = divmod(core, 4)
        s = slice(g * C, (g + 1) * C)
        in_maps.append(
            {
                "x": a(x[b]),
                "wq": a(Wq[s, :]),
                "wk": a(Wk[s, :]),
                "wv": a(Wv[s, :]),
                "wo": a(Wo[:, s]),
                "bq": a(bq[s]),
                "bk": a(bk[s]),
                "bv": a(bv[s]),
            }
        )
    return in_maps


def kernel(x, Wq, bq, Wk, bk, Wv, bv, Wo, bo, _trace=False, _trace_out=None, _tmpdir=None):
    in_maps = _make_in_maps(x, Wq, bq, Wk, bk, Wv, bv, Wo, bo)
    res = _run(in_maps, trace=_trace, tmpdir=_tmpdir)
    if _trace_out is not None:
        _trace_out.append(res)
    bo = np.asarray(bo, dtype=np.float32)
    out = np.empty((B, L, D), dtype=np.float32)
    for b in range(B):
        acc = res.results[4 * b]["out"].astype(np.float32)
        for g in range(1, 4):
            acc = acc + res.results[4 * b + g]["out"]
        out[b] = acc + bo[None, :]
    return out



# revision 2
# speedup vs baseline: 1.5395x; 1.5395x over previous
"""Multi-head causal self-attention (B=2, L=2048, D=1024, H=16) on 8 TRN2
NeuronCores.

Sharding: core c handles batch b = c // 4 and head group g = c % 4 (4 heads,
i.e. a 256-wide slice of the QKV output dim and the matching 256 rows of
Wo^T).  Each core computes a full (L, D) partial of the output projection;
the host sums the 4 partials per batch and adds bo.

v2 layout: all inputs are pre-transposed AND converted to f16 on the host
(x^T [D, L], W{q,k,v}^T [D, C], Wo_slice^T [C, D]) so no PE transposes or
PSUM->SBUF relayout casts are needed on device — phase A is pure dense
matmul.  Biases ride the PE too (rank-1 ones-row matmuls appended to each
accumulation group).  Emission interleaves the K/V/Q projections of block
qt+1 with the attention of block qt so the ACT exp stream (the co-bottleneck
at ~1.3us per [128,1024] tile) starts ~14us in and overlaps projection
matmuls.

On-core tiles (f16 unless noted):
  XT  [128, 8, 2048]   x^T (d-chunk on partitions), straight from DMA
  W*T [128, 8, 256]    W^T, straight from DMA
  WoT [128, 2, 1024]   Wo^T slice, straight from DMA
  QT  [128, 2, 512]x4  q^T (dq on partitions, chunk = head pair)
  KTz [128, 4, 512]x4  k^T zero-padded per head to K=128 rows (PE HAM clock
                       gate needs full-partition streams; K=64 is ~1.6x off)
  Vp  [128, 4, 4, 65]  v natural + ones column (softmax denominator trick)
  OT  [128, 2, 512]x4  attention out^T, normalized in place

Attention per (qt, head): s^T[k, q] = KTz_h . QT_pair; exp on ACT from a
2-bank PSUM pair; causal mask via gpsimd affine_select on diagonal tiles;
o^T + denominator accumulated in PSUM with V'; normalize = PE
ones-broadcast of the denominator + reciprocal_approx_fast (custom DVE,
~5x the iterative divide) + one multiply, emitted one tile late so the PE
stream never waits.  Output projection woven in per 512-row q block.
"""

import sys

for _p in ("/opt/trn_rl_repo", "/root/.axon_site/_ro/trn_rl_repo"):
    if _p not in sys.path:
        sys.path.append(_p)

from contextlib import ExitStack

import numpy as np

import concourse.bass as bass
import concourse.tile as tile
from concourse import bacc, mybir
from concourse.bass_utils import run_bass_kernel_spmd

F32 = mybir.dt.float32
F16 = mybir.dt.float16

B, L, D, H = 2, 2048, 1024, 16
DK = D // H  # 64
NCORES = 8
GH = 4  # heads per core
C = GH * DK  # 256: per-core slice of the qkv/head dim
QT_TILES = L // 512  # 4
DCH = D // 128  # 8


def _build_program():
    nc = bacc.Bacc("TRN2", target_bir_lowering=False, debug=False, num_devices=NCORES)

    xt_d = nc.dram_tensor("xt", [D, L], F16, kind="ExternalInput").ap()
    wqt_d = nc.dram_tensor("wqt", [D, C], F16, kind="ExternalInput").ap()
    wkt_d = nc.dram_tensor("wkt", [D, C], F16, kind="ExternalInput").ap()
    wvt_d = nc.dram_tensor("wvt", [D, C], F16, kind="ExternalInput").ap()
    wot_d = nc.dram_tensor("wot", [C, D], F16, kind="ExternalInput").ap()
    bq_d = nc.dram_tensor("bq", [C], F16, kind="ExternalInput").ap()
    bk_d = nc.dram_tensor("bk", [C], F16, kind="ExternalInput").ap()
    bv_d = nc.dram_tensor("bv", [C], F16, kind="ExternalInput").ap()
    out_d = nc.dram_tensor("out", [L, D], F16, kind="ExternalOutput").ap()

    with tile.TileContext(nc) as tc, ExitStack() as ctx:
        pool = ctx.enter_context(tc.tile_pool(name="persist", bufs=1))
        # PSUM budget (8 banks): pss [128,1024] x2 = 4 banks, pso [128,512] x2,
        # psb+psy shared tag "tmp" x2.
        psA = ctx.enter_context(tc.tile_pool(name="psA", bufs=2, space="PSUM"))
        psB = ctx.enter_context(tc.tile_pool(name="psB", bufs=2, space="PSUM"))
        cp = ctx.enter_context(tc.tile_pool(name="copies", bufs=4))
        yp = ctx.enter_context(tc.tile_pool(name="youts", bufs=3))
        rbp = ctx.enter_context(tc.tile_pool(name="rbs", bufs=2))
        dnp = ctx.enter_context(tc.tile_pool(name="dens", bufs=2))

        ones_sb = pool.tile([128, 512], F16)
        nc.gpsimd.memset(ones_sb[:], 1.0)

        bq_sb = pool.tile([1, C], F16)
        nc.sync.dma_start(bq_sb[:], bq_d[None, :])
        bk_sb = pool.tile([1, C], F16)
        nc.sync.dma_start(bk_sb[:], bk_d[None, :])
        bv_sb = pool.tile([1, C], F16)
        nc.sync.dma_start(bv_sb[:], bv_d[None, :])

        XT = pool.tile([128, DCH, L], F16)
        for dc in range(DCH):
            nc.sync.dma_start(XT[:, dc, :], xt_d[dc * 128 : (dc + 1) * 128, :])
        WT = {}
        for name, w_d in (("q", wqt_d), ("k", wkt_d), ("v", wvt_d)):
            wt = pool.tile([128, DCH, C], F16, name=f"W{name}T")
            WT[name] = wt
            nc.sync.dma_start(wt[:], w_d.rearrange("(c p) n -> p c n", p=128))
        WoT = pool.tile([128, 2, D], F16)
        nc.sync.dma_start(WoT[:], wot_d.rearrange("(c p) n -> p c n", p=128))

        QTs = [pool.tile([128, 2, 512], F16, name=f"QT{g}") for g in range(4)]
        KTzs = [pool.tile([128, GH, 512], F16, name=f"KTz{g}") for g in range(4)]
        for g in range(4):
            nc.gpsimd.memset(KTzs[g][:], 0.0)
        Vps = [pool.tile([128, 4, GH, DK + 1], F16, name=f"Vp{g}") for g in range(4)]
        for g in range(4):
            nc.vector.tensor_copy(Vps[g][:, :, :, DK : DK + 1], ones_sb[:, 0:16])
        OTs = [pool.tile([128, 2, 512], F16, name=f"OT{g}") for g in range(4)]

        with nc.allow_low_precision(reason="f16 activations/weights throughout"):

            def proj_block(qt):
                """K, V, Q projections for the 512-token block qt."""
                # ---- K^T for k-cols [512qt, 512qt+512) -> KTz (zero-padded)
                psk = psA.tile([128, 1024], F32, tag="pss", name="psk")
                for j in range(2):
                    sl = slice(j * 512, (j + 1) * 512)
                    for dci in range(DCH):
                        nc.tensor.matmul(
                            psk[:, sl],
                            lhsT=WT["k"][:, dci, j * 128 : (j + 1) * 128],
                            rhs=XT[:, dci, qt * 512 : (qt + 1) * 512],
                            start=(dci == 0),
                            stop=False,
                        )
                    nc.tensor.matmul(
                        psk[:, sl],
                        lhsT=bk_sb[0:1, j * 128 : (j + 1) * 128],
                        rhs=ones_sb[0:1, 0:512],
                        start=False,
                        stop=True,
                    )
                for j in range(2):
                    for half in range(2):
                        hp = 64 * half
                        nc.vector.tensor_copy(
                            KTzs[qt][hp : hp + 64, 2 * j + half, :],
                            psk[hp : hp + 64, j * 512 : (j + 1) * 512],
                        )
                # ---- V natural for the same k-block -> Vp
                psv = psA.tile([128, 1024], F32, tag="pss", name="psv")
                for kk in range(4):
                    kt = 4 * qt + kk
                    sl = slice(kk * 256, (kk + 1) * 256)
                    for dci in range(DCH):
                        nc.tensor.matmul(
                            psv[:, sl],
                            lhsT=XT[:, dci, kt * 128 : (kt + 1) * 128],
                            rhs=WT["v"][:, dci, :],
                            start=(dci == 0),
                            stop=False,
                        )
                    nc.tensor.matmul(
                        psv[:, sl],
                        lhsT=ones_sb[0:1, 0:128],
                        rhs=bv_sb[:],
                        start=False,
                        stop=True,
                    )
                nc.vector.tensor_copy(
                    Vps[qt][:, :, :, 0:DK],
                    psv[:].rearrange("p (k h d) -> p k h d", k=4, h=GH),
                )
                # ---- Q^T for q-rows [512qt, 512qt+512) -> QT
                psq = psA.tile([128, 1024], F32, tag="pss", name="psq")
                for j in range(2):
                    sl = slice(j * 512, (j + 1) * 512)
                    for dci in range(DCH):
                        nc.tensor.matmul(
                            psq[:, sl],
                            lhsT=WT["q"][:, dci, j * 128 : (j + 1) * 128],
                            rhs=XT[:, dci, qt * 512 : (qt + 1) * 512],
                            start=(dci == 0),
                            stop=False,
                        )
                    nc.tensor.matmul(
                        psq[:, sl],
                        lhsT=bq_sb[0:1, j * 128 : (j + 1) * 128],
                        rhs=ones_sb[0:1, 0:512],
                        start=False,
                        stop=True,
                    )
                nc.vector.tensor_copy(
                    QTs[qt][:], psq[:].rearrange("p (j n) -> p j n", j=2)
                )

            def normalize(h, qt, pso):
                hj, hp = h // 2, 64 * (h % 2)
                den_r = dnp.tile([1, 512], F16, tag="den")
                nc.vector.tensor_copy(den_r[:], pso[64:65, :])
                psb = psB.tile([128, 512], F32, tag="tmp", name="psb")
                nc.tensor.matmul(
                    psb[:64],
                    lhsT=ones_sb[0:1, 0:64],
                    rhs=den_r[:],
                    start=True,
                    stop=True,
                )
                rb = rbp.tile([64, 512], F32, tag="rb")
                nc.vector.reciprocal_approx_fast(rb[:], psb[:64])
                nc.vector.tensor_tensor(
                    OTs[qt][hp : hp + 64, hj, :],
                    pso[:64],
                    rb[:],
                    mybir.AluOpType.mult,
                )

            def outproj(qt512):
                # project q rows [qt512*512, +512) and DMA them out; woven
                # into the next q-tile's attention so PE stays dense and the
                # output DMA is spread across the kernel.
                for sub in range(4):
                    q0 = qt512 * 512 + sub * 128
                    for e in range(2):
                        psy = psB.tile([128, 512], F32, tag="tmp", name="psy")
                        for cj in range(2):
                            nc.tensor.matmul(
                                psy[:],
                                lhsT=OTs[qt512][:, cj, sub * 128 : (sub + 1) * 128],
                                rhs=WoT[:, cj, e * 512 : (e + 1) * 512],
                                start=(cj == 0),
                                stop=(cj == 1),
                            )
                        y_sb = yp.tile([128, 512], F16, tag="y")
                        nc.vector.tensor_copy(y_sb[:], psy[:])
                        nc.sync.dma_start(
                            out_d[q0 : q0 + 128, e * 512 : (e + 1) * 512], y_sb[:]
                        )

            pending = None
            for qt in range(QT_TILES):
                proj_block(qt)
                n_kt = 4 * qt + 4
                for h in range(GH):
                    hj = h // 2
                    pso = psB.tile([128, 512], F32, tag="pso")
                    for g in range(n_kt // 2):  # k-tile pairs share a 2-bank psum
                        pss = psA.tile([128, 1024], F32, tag="pss", name="pss")
                        for i in range(2):
                            kt = 2 * g + i
                            nc.tensor.matmul(
                                pss[:, i * 512 : (i + 1) * 512],
                                lhsT=KTzs[kt // 4][
                                    :, h, (kt % 4) * 128 : (kt % 4 + 1) * 128
                                ],
                                rhs=QTs[qt][:, hj, :],
                                start=True,
                                stop=True,
                            )
                        p_sb = cp.tile([128, 1024], F16, tag="p", bufs=5)
                        nc.scalar.activation(
                            p_sb[:],
                            pss[:],
                            mybir.ActivationFunctionType.Exp,
                            scale=0.125,
                        )
                        for i in range(2):
                            kt = 2 * g + i
                            if kt >= 4 * qt:  # diagonal overlap: causal mask
                                nc.gpsimd.affine_select(
                                    out=p_sb[:, i * 512 : (i + 1) * 512],
                                    in_=p_sb[:, i * 512 : (i + 1) * 512],
                                    pattern=[[1, 512]],
                                    compare_op=mybir.AluOpType.is_ge,
                                    fill=0.0,
                                    base=qt * 512 - kt * 128,
                                    channel_multiplier=-1,
                                )
                            nc.tensor.matmul(
                                pso[:65],
                                lhsT=Vps[kt // 4][:, kt % 4, h, :],
                                rhs=p_sb[:, i * 512 : (i + 1) * 512],
                                start=(kt == 0),
                                stop=(kt == n_kt - 1),
                            )
                        if g == 0 and pending is not None:
                            normalize(*pending)  # previous tile, PE has work
                            pending = None
                    pending = (h, qt, pso)
                    if h == 0 and qt > 0:
                        outproj(qt - 1)  # previous q block fully normalized
            normalize(*pending)
            outproj(QT_TILES - 1)

    nc.compile()
    return nc


_NC_CACHE = None


def _get_program():
    global _NC_CACHE
    if _NC_CACHE is None:
        _NC_CACHE = _build_program()
    return _NC_CACHE


def _run(in_maps, trace=False, **kw):
    nc = _get_program()
    return run_bass_kernel_spmd(nc, in_maps, list(range(NCORES)), trace=trace, **kw)


def _make_in_maps(x, Wq, bq, Wk, bk, Wv, bv, Wo, bo):
    f16 = lambda v: np.ascontiguousarray(np.asarray(v, dtype=np.float32)).astype(
        np.float16
    )
    x = np.asarray(x, dtype=np.float32)
    in_maps = []
    for core in range(NCORES):
        b, g = divmod(core, 4)
        s = slice(g * C, (g + 1) * C)
        in_maps.append(
            {
                "xt": f16(x[b].T),
                "wqt": f16(Wq[s, :].T),
                "wkt": f16(Wk[s, :].T),
                "wvt": f16(Wv[s, :].T),
                "wot": f16(Wo[:, s].T),
                "bq": f16(bq[s]),
                "bk": f16(bk[s]),
                "bv": f16(bv[s]),
            }
        )
    return in_maps


def kernel(x, Wq, bq, Wk, bk, Wv, bv, Wo, bo, _trace=False, _trace_out=None, _tmpdir=None):
    in_maps = _make_in_maps(x, Wq, bq, Wk, bk, Wv, bv, Wo, bo)
    res = _run(in_maps, trace=_trace, tmpdir=_tmpdir)
    if _trace_out is not None:
        _trace_out.append(res)
    bo = np.asarray(bo, dtype=np.float32)
    out = np.empty((B, L, D), dtype=np.float32)
    for b in range(B):
        acc = res.results[4 * b]["out"].astype(np.float32)
        for g in range(1, 4):
            acc = acc + res.results[4 * b + g]["out"].astype(np.float32)
        out[b] = acc + bo[None, :]
    return out
